# revision 7
# baseline (speedup 1.0000x reference)
"""Causal self-attention kernel for Trainium2, 8-core SPMD — transfer- and
instruction-count-optimized.

Problem: B=4, L=2048, D=768, H=12 heads (hd=64); y = attn(x) @ w_proj + b_proj.

Wall-clock per call = fixed dispatch (~180ms) + upload (~6ms/MB) + output
(~14ms/MB zeros+readback) + device execution, where execution on this stack
costs ~50-80us PER INSTRUCTION regardless of size. The design therefore
minimizes both bytes moved and instruction count:

- All device inputs bf16 (error gate 2e-2; bf16 end-to-end lands ~5e-3).
- The host uploads x TRANSPOSED (xT), split per pair: core c gets columns
  [1024*(c%2):...] of batch c//2's xT; one pair AllGather reconstructs it.
  This removes all 192 on-device PE-transpose+copy instructions.
- Weights ship as ONE tensor W_comb = [wqkv_g (768x1152); wp_g padded
  (384x1152)] sharded 4-ways by row, one quad AllGather -> full weights.
- No attention AllGather: each core computes a row-parallel PARTIAL
  projection from its 6 heads into pp_all [2048,768]; ONE pairwise
  ReduceScatter(add) gives each core its half of the rows (even core: rows
  0:1024). Output [1024,768] bf16/core; host concatenates, casts, adds bias.
- Scores are staged psum->sbuf with cheap vector copies and exp'd in wide
  [128, T*512] batches (ACT cost is width-independent).
- Constants (identity, causal mask, v-ones) are generated on device.

Per-call: ~18MB up + 12.6MB zeros+readback, ~1100 device instructions.

Core c: batch b=c//2, head-group g=c%2 (6 heads). Flash-style layout:
scores transposed (keys on partitions), ones-augmented V row for softmax
denominators, bf16 matmuls with f32 PSUM.
"""

import numpy as np
import ml_dtypes

import concourse.bacc as bacc
import concourse.mybir as mybir
import concourse.tile as tile
from concourse.bass_utils import run_bass_kernel_spmd

F32 = mybir.dt.float32
BF16 = mybir.dt.bfloat16

B, L, D = 4, 2048, 768
NHEAD = 12
HD = 64
NH = 6              # local heads per core
HDL = NH * HD       # 384: local head dims
KD = D // 128       # 6 D-tiles
NQ = 4              # q chunks
QW = L // NQ        # 512: q chunk width
HL = L // 2         # 1024: x cols uploaded per core / output rows per core
WR = D + HDL        # 1152: W_comb rows (768 wqkv + 384 wp)
NEG = -1.0e30

PAIRS = [[0, 1], [2, 3], [4, 5], [6, 7]]
QUADS = [[0, 2, 4, 6], [1, 3, 5, 7]]

_CACHED_NC = None


def build_nc():
    nc = bacc.Bacc(None, num_devices=8, debug=False)

    xt_d = nc.dram_tensor("xt", [D, HL], BF16, kind="ExternalInput")
    w_d = nc.dram_tensor("w", [WR // 4, WR], BF16, kind="ExternalInput")
    out_d = nc.dram_tensor("out", [HL, D], BF16, kind="ExternalOutput")

    # collectives cannot touch IO tensors: stage through Internal DRAM
    xt_i = nc.dram_tensor("xt_i", [D, HL], BF16, kind="Internal")
    w_i = nc.dram_tensor("w_i", [WR // 4, WR], BF16, kind="Internal")
    xtf_d = nc.dram_tensor("xtf", [2 * D, HL], BF16, kind="Internal")
    wf_d = nc.dram_tensor("wf", [WR, WR], BF16, kind="Internal")
    pp_d = nc.dram_tensor("pp", [L, D], BF16, kind="Internal")
    ro_d = nc.dram_tensor("ro", [HL, D], BF16, kind="Internal")

    with tile.TileContext(nc) as tc:
        with (
            tc.tile_pool(name="persist", bufs=1) as pers,
            tc.tile_pool(name="attn", bufs=1) as attn_pool,
            tc.tile_pool(name="work", bufs=2) as work,
            tc.tile_pool(name="psum", bufs=2, space="PSUM") as pp,
        ):
            # ---------------- Phase 0a: input AllGathers ---------------------
            nc.sync.dma_start(xt_i[:], xt_d[:])
            nc.sync.dma_start(w_i[:], w_d[:])
            nc.gpsimd.collective_compute(
                "AllGather", mybir.AluOpType.bypass,
                replica_groups=PAIRS, ins=[xt_i[:]], outs=[xtf_d[:]],
            )
            nc.gpsimd.collective_compute(
                "AllGather", mybir.AluOpType.bypass,
                replica_groups=QUADS, ins=[w_i[:]], outs=[wf_d[:]],
            )

            # ---------------- Phase 0b: constants generated on device --------
            tmp1 = pers.tile([128, 128], BF16)
            nc.vector.memset(tmp1[:], 1.0)
            identb = pers.tile([128, 128], BF16)
            # identb[p,f] = 1 if f==p else 0
            nc.gpsimd.affine_select(
                identb[:], tmp1[:], pattern=[[1, 128]],
                compare_op=mybir.AluOpType.is_equal, fill=0.0,
                channel_multiplier=-1,
            )
            tmp0 = pers.tile([128, 128], BF16)
            nc.vector.memset(tmp0[:], 0.0)
            nmask = pers.tile([128, 128], BF16)
            # nmask[kp,qf] = 0 if qf >= kp else -1e30
            nc.gpsimd.affine_select(
                nmask[:], tmp0[:], pattern=[[1, 128]],
                compare_op=mybir.AluOpType.is_ge, fill=NEG,
                channel_multiplier=-1,
            )

            # ---------------- Phase 0c: weights + xT to SBUF -----------------
            wqk = pers.tile([128, KD, 2 * HDL], BF16)
            nc.sync.dma_start(
                wqk[:], wf_d[0:D, 0 : 2 * HDL].rearrange("(a p) n -> p a n", p=128)
            )
            wv = pers.tile([128, KD, HDL], BF16)
            nc.sync.dma_start(
                wv[:],
                wf_d[0:D, 2 * HDL : 3 * HDL].rearrange("(a p) n -> p a n", p=128),
            )
            wp = pers.tile([128, 3, D], BF16)
            nc.sync.dma_start(
                wp[:], wf_d[D : D + HDL, 0:D].rearrange("(a p) n -> p a n", p=128)
            )

            xT = [attn_pool.tile([128, L], BF16, name=f"xT{j}") for j in range(KD)]
            for j in range(KD):
                nc.sync.dma_start(
                    xT[j][:, 0:HL], xtf_d[128 * j : 128 * (j + 1), :]
                )
                nc.sync.dma_start(
                    xT[j][:, HL:L], xtf_d[D + 128 * j : D + 128 * (j + 1), :]
                )

            # ---------------- Phase 1: kqT = (x @ wqk).T ---------------------
            # kqT[m] [128, L]; m=0..2: qT head pairs; m=3..5: kT head pairs
            kqT = [attn_pool.tile([128, L], BF16, name=f"kqT{m}") for m in range(6)]
            for m in range(6):
                for qc in range(L // 512):
                    pt = pp.tile([128, 512], F32, tag="psA", bufs=2)
                    for j in range(KD):
                        nc.tensor.matmul(
                            pt[:],
                            wqk[:, j, 128 * m : 128 * (m + 1)],
                            xT[j][:, 512 * qc : 512 * (qc + 1)],
                            start=(j == 0),
                            stop=(j == KD - 1),
                        )
                    nc.vector.tensor_copy(kqT[m][:, 512 * qc : 512 * (qc + 1)], pt[:])

            # ---------------- Phase 2: v_aug [128, L/128, NH*65] -------------
            v_sb = attn_pool.tile([128, L // 128, NH * 65], BF16)
            nc.vector.memset(
                v_sb[:].rearrange("p a (h w) -> p a h w", h=NH)[:, :, :, 64:65], 1.0
            )
            for i in range(L // 128):
                pv = pp.tile([128, HDL], F32, tag="psA", bufs=2)
                for j in range(KD):
                    nc.tensor.matmul(
                        pv[:],
                        xT[j][:, 128 * i : 128 * (i + 1)],
                        wv[:, j, :],
                        start=(j == 0),
                        stop=(j == KD - 1),
                    )
                nc.vector.tensor_copy(
                    v_sb[:].rearrange("p a (h w) -> p a h w", h=NH)[:, i, :, 0:64],
                    pv[:].rearrange("p (h w) -> p h w", h=NH),
                )

            # ------ Phases 3-4, chunked over q: attention -> partial proj ----
            late_ctx = tc.tile_pool(name="late", bufs=1)
            late = late_ctx.__enter__()

            for qq in range(NQ):
                q0 = QW * qq
                q1 = q0 + QW
                T = q1 // 128  # key tiles in play for this chunk
                # aoT[j]: normalized attention output rows for local heads
                # (2j, 2j+1), this chunk's 512 q columns.
                aoT = [
                    late.tile([128, QW], BF16, tag="aoT", bufs=6, name=f"aoT{qq}_{j}")
                    for j in range(3)
                ]
                for h in range(NH):
                    p, sub = h // 2, h % 2
                    qT_h = kqT[p]
                    kT_h = kqT[3 + p]
                    # scores for all T key tiles, staged to SBUF in bf16
                    sc = late.tile([128, T * 512], BF16, tag="sc", bufs=2,
                                   name=f"sc{qq}_{h}")
                    for t in range(T):
                        qs = max(128 * t, q0)
                        W = q1 - qs
                        sp = pp.tile([128, QW], F32, tag="sp", bufs=2)
                        diag = 128 * t >= q0
                        nc.tensor.matmul(
                            sp[:, :W],
                            kT_h[64 * sub : 64 * sub + 64, 128 * t : 128 * (t + 1)],
                            qT_h[64 * sub : 64 * sub + 64, qs:q1],
                            start=True,
                            stop=not diag,
                            tile_position=(64 * sub, 0),
                        )
                        if diag:
                            # add causal mask into the diagonal block via PE:
                            # sp[:, :128] += ident.T @ nmask
                            nc.tensor.matmul(
                                sp[:, 0:128],
                                identb[:],
                                nmask[:],
                                start=False,
                                stop=True,
                            )
                        nc.vector.tensor_copy(
                            sc[:, 512 * t : 512 * t + W], sp[:, :W]
                        )
                    # wide exp batches (ACT cost is width-independent)
                    ex = late.tile([128, T * 512], BF16, tag="ex", bufs=2,
                                   name=f"ex{qq}_{h}")
                    for eb in range(T // 4):
                        nc.scalar.activation(
                            ex[:, 2048 * eb : 2048 * (eb + 1)],
                            sc[:, 2048 * eb : 2048 * (eb + 1)],
                            mybir.ActivationFunctionType.Exp,
                            scale=0.125,
                        )
                    # attn @ v_aug, accumulated over key tiles
                    oa = pp.tile([65, QW], F32, tag="oa", bufs=2)
                    for t in range(T):
                        qs = max(128 * t, q0)
                        W = q1 - qs
                        nc.tensor.matmul(
                            oa[:, qs - q0 :],
                            v_sb[:, t, 65 * h : 65 * h + 65],
                            ex[:, 512 * t : 512 * t + W],
                            start=(t == 0),
                            stop=(t == T - 1),
                        )
                    # normalize: aoT slice = oa[0:64] / oa[64]
                    aou = late.tile([65, QW], F32, tag="aou", bufs=3)
                    nc.vector.tensor_copy(aou[:], oa[:])
                    r0 = late.tile([1, QW], F32, tag="r0", bufs=3)
                    nc.vector.tensor_copy(r0[:], aou[64:65, :])
                    rdb = late.tile([64, QW], F32, tag="rdb", bufs=3)
                    nc.gpsimd.partition_broadcast(rdb[:], r0[:])
                    nc.vector.reciprocal(rdb[:], rdb[:])
                    nc.gpsimd.tensor_mul(
                        out=aoT[p][64 * sub : 64 * sub + 64, :],
                        in0=aou[0:64, :],
                        in1=rdb[:],
                    )
                # partial projection for this chunk's 4 row tiles
                for i in range(QW // 128):
                    osb = late.tile([128, D], BF16, tag="osb", bufs=3)
                    for nchunk in range(2):
                        ns = 384 * nchunk
                        po = pp.tile([128, 384], F32, tag="psA", bufs=2)
                        for j in range(3):
                            nc.tensor.matmul(
                                po[:],
                                aoT[j][:, 128 * i : 128 * (i + 1)],
                                wp[:, j, ns : ns + 384],
                                start=(j == 0),
                                stop=(j == 2),
                            )
                        nc.vector.tensor_copy(osb[:, ns : ns + 384], po[:])
                    nc.sync.dma_start(
                        pp_d[q0 + 128 * i : q0 + 128 * (i + 1), :], osb[:]
                    )

            # one pairwise ReduceScatter sums the head-group partials and
            # leaves this core with its half of the batch's output rows
            nc.gpsimd.collective_compute(
                "ReduceScatter", mybir.AluOpType.add,
                replica_groups=PAIRS, ins=[pp_d[:]], outs=[ro_d[:]],
            )
            nc.sync.dma_start(out_d[:], ro_d[:])
            late_ctx.__exit__(None, None, None)

    nc.compile()
    return nc


def get_nc():
    global _CACHED_NC
    if _CACHED_NC is None:
        _CACHED_NC = build_nc()
    return _CACHED_NC


def make_in_maps(x, w_attn, w_proj, b_proj):
    bf16 = ml_dtypes.bfloat16
    x = np.asarray(x, dtype=np.float32)
    w_attn = np.asarray(w_attn, dtype=np.float32)
    w_proj = np.asarray(w_proj, dtype=np.float32)

    # x.T per batch, bf16: [768, 2048] each
    xtb = [np.ascontiguousarray(x[b].T).astype(bf16) for b in range(B)]
    # W_comb per head-group: rows 0:768 = [q|k|v] columns of this group,
    # rows 768:1152 = this group's w_proj rows padded to 1152 cols
    wq, wk, wv_ = (w_attn[:, k * D : (k + 1) * D] for k in range(3))
    wcomb_g = []
    for g in range(2):
        cols = slice(HDL * g, HDL * (g + 1))
        top = np.concatenate([wq[:, cols], wk[:, cols], wv_[:, cols]], axis=1)
        bot = np.zeros((HDL, WR), np.float32)
        bot[:, 0:D] = w_proj[HDL * g : HDL * (g + 1)]
        wcomb_g.append(np.concatenate([top, bot], axis=0).astype(bf16))

    in_maps = []
    for c in range(8):
        b, g, r = c // 2, c % 2, c // 2
        in_maps.append(
            {
                "xt": xtb[b][:, HL * g : HL * (g + 1)],
                "w": wcomb_g[g][288 * r : 288 * (r + 1)],
            }
        )
    return in_maps


def kernel(x, w_attn, w_proj, b_proj):
    nc = get_nc()
    in_maps = make_in_maps(x, w_attn, w_proj, b_proj)
    res = run_bass_kernel_spmd(nc, in_maps, core_ids=list(range(8)))
    out = np.empty((B, L, D), np.float32)
    for b in range(B):
        out[b, 0:HL] = np.asarray(res.results[2 * b]["out"])
        out[b, HL:L] = np.asarray(res.results[2 * b + 1]["out"])
    out += np.asarray(b_proj, dtype=np.float32)
    return out


# revision 8
# speedup vs baseline: 1.1403x; 1.1403x over previous
"""Causal self-attention kernel for Trainium2, 8-core SPMD — transfer- and
instruction-count-optimized, engine-pipelined.

Problem: B=4, L=2048, D=768, H=12 heads (hd=64); y = attn(x) @ w_proj + b_proj.

Wall-clock per call = fixed dispatch (~180ms) + upload (~6ms/MB) + output
(~14ms/MB zeros+readback) + device execution. Execution on this stack costs
~50-80us PER INSTRUCTION (size-independent) on per-engine in-order queues, so
the design minimizes bytes moved, minimizes instruction count on the busiest
engine (PE), and keeps cross-engine consumers a full head behind producers so
PE never stalls:

- All device inputs bf16 (error gate 2e-2; bf16 end-to-end lands ~5e-3).
- The host uploads x TRANSPOSED (xT), split per pair: core c gets columns
  [1024*(c%2):...] of batch c//2's xT; one pair AllGather reconstructs it
  (no on-device transposes at all).
- Weights ship as ONE tensor W_comb = [wqkv_g (768x1152); wp_g padded
  (384x1152)] sharded 4-ways by row; one quad AllGather -> full weights.
- No attention AllGather: each core computes a row-parallel PARTIAL
  projection from its 6 heads into pp [2048,768]; ONE pairwise
  ReduceScatter(add) leaves each core its half of the rows (even core: rows
  0:1024). Output [1024,768] bf16/core; host concatenates, casts, adds bias.
- Scores are staged psum->sbuf with cheap vector copies and exp'd in wide
  [128, T*512] batches (ACT cost is width-independent). attn@v for head h is
  emitted under head h+1's score matmuls; projection tiles of the previous
  chunk fill PE between heads.
- Causal diag masking via PE mask-add matmuls (ident.T @ nmask into psum).
- Constants (identity, causal mask, v-ones) are generated on device.

Core c: batch b=c//2, head-group g=c%2 (6 heads). Transposed-scores layout
(keys on partitions), ones-augmented V row for softmax denominators, bf16
matmuls with f32 PSUM.
"""

import numpy as np
import ml_dtypes

import concourse.bacc as bacc
import concourse.mybir as mybir
import concourse.tile as tile
from concourse.bass_utils import run_bass_kernel_spmd

F32 = mybir.dt.float32
BF16 = mybir.dt.bfloat16

B, L, D = 4, 2048, 768
NHEAD = 12
HD = 64
NH = 6              # local heads per core
HDL = NH * HD       # 384: local head dims
KD = D // 128       # 6 D-tiles
NQ = 4              # q chunks
QW = L // NQ        # 512: q chunk width
HL = L // 2         # 1024: x cols uploaded per core / output rows per core
WR = D + HDL        # 1152: W_comb rows (768 wqkv + 384 wp)
NEG = -1.0e30

PAIRS = [[0, 1], [2, 3], [4, 5], [6, 7]]
QUADS = [[0, 2, 4, 6], [1, 3, 5, 7]]

_CACHED_NC = None


def build_nc():
    nc = bacc.Bacc(None, num_devices=8, debug=False)

    xt_d = nc.dram_tensor("xt", [D, HL], BF16, kind="ExternalInput")
    w_d = nc.dram_tensor("w", [WR // 4, WR], BF16, kind="ExternalInput")
    out_d = nc.dram_tensor("out", [HL, D], BF16, kind="ExternalOutput")

    # collectives cannot touch IO tensors: stage through Internal DRAM
    xt_i = nc.dram_tensor("xt_i", [D, HL], BF16, kind="Internal")
    w_i = nc.dram_tensor("w_i", [WR // 4, WR], BF16, kind="Internal")
    xtf_d = nc.dram_tensor("xtf", [2 * D, HL], BF16, kind="Internal")
    wf_d = nc.dram_tensor("wf", [WR, WR], BF16, kind="Internal")
    pp_d = nc.dram_tensor("pp", [L, D], BF16, kind="Internal")
    ro_d = nc.dram_tensor("ro", [HL, D], BF16, kind="Internal")

    with tile.TileContext(nc) as tc:
        with (
            tc.tile_pool(name="persist", bufs=1) as pers,
            tc.tile_pool(name="attn", bufs=1) as attn_pool,
            tc.tile_pool(name="psum", bufs=2, space="PSUM") as pp,
        ):
            # ---------------- Phase 0a: input AllGathers ---------------------
            nc.sync.dma_start(xt_i[:], xt_d[:])
            nc.sync.dma_start(w_i[:], w_d[:])
            nc.gpsimd.collective_compute(
                "AllGather", mybir.AluOpType.bypass,
                replica_groups=PAIRS, ins=[xt_i[:]], outs=[xtf_d[:]],
            )
            nc.gpsimd.collective_compute(
                "AllGather", mybir.AluOpType.bypass,
                replica_groups=QUADS, ins=[w_i[:]], outs=[wf_d[:]],
            )

            # ---------------- Phase 0b: constants generated on device --------
            tmp1 = pers.tile([128, 128], BF16)
            nc.vector.memset(tmp1[:], 1.0)
            identb = pers.tile([128, 128], BF16)
            # identb[p,f] = 1 if f==p else 0
            nc.gpsimd.affine_select(
                identb[:], tmp1[:], pattern=[[1, 128]],
                compare_op=mybir.AluOpType.is_equal, fill=0.0,
                channel_multiplier=-1,
            )
            tmp0 = pers.tile([128, 128], BF16)
            nc.vector.memset(tmp0[:], 0.0)
            nmask = pers.tile([128, 128], BF16)
            # nmask[kp,qf] = 0 if qf >= kp else -1e30
            nc.gpsimd.affine_select(
                nmask[:], tmp0[:], pattern=[[1, 128]],
                compare_op=mybir.AluOpType.is_ge, fill=NEG,
                channel_multiplier=-1,
            )

            # ---------------- Phase 0c: weights + xT to SBUF -----------------
            wqk = pers.tile([128, KD, 2 * HDL], BF16)
            nc.sync.dma_start(
                wqk[:], wf_d[0:D, 0 : 2 * HDL].rearrange("(a p) n -> p a n", p=128)
            )
            wv = pers.tile([128, KD, HDL], BF16)
            nc.sync.dma_start(
                wv[:],
                wf_d[0:D, 2 * HDL : 3 * HDL].rearrange("(a p) n -> p a n", p=128),
            )
            wp = pers.tile([128, 3, D], BF16)
            nc.sync.dma_start(
                wp[:], wf_d[D : D + HDL, 0:D].rearrange("(a p) n -> p a n", p=128)
            )

            xT = [attn_pool.tile([128, L], BF16, name=f"xT{j}") for j in range(KD)]
            for j in range(KD):
                nc.sync.dma_start(
                    xT[j][:, 0:HL], xtf_d[128 * j : 128 * (j + 1), :]
                )
                nc.sync.dma_start(
                    xT[j][:, HL:L], xtf_d[D + 128 * j : D + 128 * (j + 1), :]
                )

            # ---------------- Phase 1: kqT = (x @ wqk).T ---------------------
            # kqT[m] [128, L]; m=0..2: qT head pairs; m=3..5: kT head pairs
            kqT = [attn_pool.tile([128, L], BF16, name=f"kqT{m}") for m in range(6)]
            for m in range(6):
                for qc in range(L // 512):
                    pt = pp.tile([128, 512], F32, tag="psA", bufs=2)
                    for j in range(KD):
                        nc.tensor.matmul(
                            pt[:],
                            wqk[:, j, 128 * m : 128 * (m + 1)],
                            xT[j][:, 512 * qc : 512 * (qc + 1)],
                            start=(j == 0),
                            stop=(j == KD - 1),
                        )
                    nc.vector.tensor_copy(kqT[m][:, 512 * qc : 512 * (qc + 1)], pt[:])

            # ---------------- Phase 2: v_aug [128, L/128, NH*65] -------------
            v_sb = attn_pool.tile([128, L // 128, NH * 65], BF16)
            nc.vector.memset(
                v_sb[:].rearrange("p a (h w) -> p a h w", h=NH)[:, :, :, 64:65], 1.0
            )
            for i in range(L // 128):
                pv = pp.tile([128, HDL], F32, tag="psA", bufs=2)
                for j in range(KD):
                    nc.tensor.matmul(
                        pv[:],
                        xT[j][:, 128 * i : 128 * (i + 1)],
                        wv[:, j, :],
                        start=(j == 0),
                        stop=(j == KD - 1),
                    )
                nc.vector.tensor_copy(
                    v_sb[:].rearrange("p a (h w) -> p a h w", h=NH)[:, i, :, 0:64],
                    pv[:].rearrange("p (h w) -> p h w", h=NH),
                )

            # ------ Phases 3-4, chunked over q: attention -> partial proj ----
            late_ctx = tc.tile_pool(name="late", bufs=1)
            late = late_ctx.__enter__()

            def emit_proj_tile(qq, aoT, i):
                # one 128-row tile of this core's PARTIAL projection
                q0 = QW * qq
                osb = late.tile([128, D], BF16, tag="osb", bufs=3)
                for nchunk in range(2):
                    ns = 384 * nchunk
                    po = pp.tile([128, 384], F32, tag="psA", bufs=2)
                    for j in range(3):
                        nc.tensor.matmul(
                            po[:],
                            aoT[j][:, 128 * i : 128 * (i + 1)],
                            wp[:, j, ns : ns + 384],
                            start=(j == 0),
                            stop=(j == 2),
                        )
                    nc.vector.tensor_copy(osb[:, ns : ns + 384], po[:])
                nc.sync.dma_start(
                    pp_d[q0 + 128 * i : q0 + 128 * (i + 1), :], osb[:]
                )

            proj_queue = []

            for qq in range(NQ):
                q0 = QW * qq
                q1 = q0 + QW
                T = q1 // 128  # key tiles in play for this chunk
                chunk_ctx = tc.tile_pool(name=f"chunk{qq}", bufs=1)
                cpool = chunk_ctx.__enter__()
                # aoT[j]: normalized attention output rows for local heads
                # (2j, 2j+1), this chunk's 512 q columns (outlives the chunk).
                aoT = [
                    late.tile([128, QW], BF16, tag="aoT", bufs=6, name=f"aoT{qq}_{j}")
                    for j in range(3)
                ]

                def emit_attnv_norm(h, ex):
                    # attn @ v_aug for head h (reads ex staged one head ago)
                    p, sub = h // 2, h % 2
                    oa = pp.tile([65, QW], F32, tag="oa", bufs=2)
                    for t in range(T):
                        qs = max(128 * t, q0)
                        W = q1 - qs
                        nc.tensor.matmul(
                            oa[:, qs - q0 :],
                            v_sb[:, t, 65 * h : 65 * h + 65],
                            ex[:, 512 * t : 512 * t + W],
                            start=(t == 0),
                            stop=(t == T - 1),
                        )
                    # normalize: aoT slice = oa[0:64] / oa[64]
                    aou = late.tile([65, QW], F32, tag="aou", bufs=3)
                    nc.vector.tensor_copy(aou[:], oa[:])
                    r0 = late.tile([1, QW], F32, tag="r0", bufs=3)
                    nc.vector.tensor_copy(r0[:], aou[64:65, :])
                    rdb = late.tile([64, QW], F32, tag="rdb", bufs=3)
                    nc.gpsimd.partition_broadcast(rdb[:], r0[:])
                    nc.vector.reciprocal(rdb[:], rdb[:])
                    nc.gpsimd.tensor_mul(
                        out=aoT[p][64 * sub : 64 * sub + 64, :],
                        in0=aou[0:64, :],
                        in1=rdb[:],
                    )

                prev = None
                for h in range(NH):
                    p, sub = h // 2, h % 2
                    qT_h = kqT[p]
                    kT_h = kqT[3 + p]
                    # scores for all T key tiles, staged to SBUF in bf16
                    sc = cpool.tile([128, T * 512], BF16, tag="sc", bufs=2,
                                    name=f"sc{qq}_{h % 2}")
                    for t in range(T):
                        qs = max(128 * t, q0)
                        W = q1 - qs
                        sp = pp.tile([128, QW], F32, tag="sp", bufs=4)
                        diag = 128 * t >= q0
                        nc.tensor.matmul(
                            sp[:, :W],
                            kT_h[64 * sub : 64 * sub + 64, 128 * t : 128 * (t + 1)],
                            qT_h[64 * sub : 64 * sub + 64, qs:q1],
                            start=True,
                            stop=not diag,
                            tile_position=(64 * sub, 0),
                        )
                        if diag:
                            # add causal mask into the diagonal block via PE:
                            # sp[:, :128] += ident.T @ nmask
                            nc.tensor.matmul(
                                sp[:, 0:128],
                                identb[:],
                                nmask[:],
                                start=False,
                                stop=True,
                            )
                        nc.vector.tensor_copy(
                            sc[:, 512 * t : 512 * t + W], sp[:, :W]
                        )
                    # wide exp batches (ACT cost is width-independent)
                    ex = cpool.tile([128, T * 512], BF16, tag="ex", bufs=2,
                                    name=f"ex{qq}_{h % 2}")
                    for eb in range(T // 4):
                        nc.scalar.activation(
                            ex[:, 2048 * eb : 2048 * (eb + 1)],
                            sc[:, 2048 * eb : 2048 * (eb + 1)],
                            mybir.ActivationFunctionType.Exp,
                            scale=0.125,
                        )
                    # PE filler while this head's exp runs: previous head's
                    # attn@v, then a projection tile of the previous chunk
                    if prev is not None:
                        emit_attnv_norm(*prev)
                    if h >= 2 and proj_queue:
                        emit_proj_tile(*proj_queue.pop(0))
                    prev = (h, ex)
                emit_attnv_norm(*prev)
                proj_queue.extend((qq, aoT, i) for i in range(QW // 128))
                chunk_ctx.__exit__(None, None, None)

            while proj_queue:
                emit_proj_tile(*proj_queue.pop(0))

            # one pairwise ReduceScatter sums the head-group partials and
            # leaves this core with its half of the batch's output rows
            nc.gpsimd.collective_compute(
                "ReduceScatter", mybir.AluOpType.add,
                replica_groups=PAIRS, ins=[pp_d[:]], outs=[ro_d[:]],
            )
            nc.sync.dma_start(out_d[:], ro_d[:])
            late_ctx.__exit__(None, None, None)

    nc.compile()
    return nc


def get_nc():
    global _CACHED_NC
    if _CACHED_NC is None:
        _CACHED_NC = build_nc()
    return _CACHED_NC


def make_in_maps(x, w_attn, w_proj, b_proj):
    bf16 = ml_dtypes.bfloat16
    x = np.asarray(x, dtype=np.float32)
    w_attn = np.asarray(w_attn, dtype=np.float32)
    w_proj = np.asarray(w_proj, dtype=np.float32)

    # x.T per batch, bf16: [768, 2048] each
    xtb = [np.ascontiguousarray(x[b].T).astype(bf16) for b in range(B)]
    # W_comb per head-group: rows 0:768 = [q|k|v] columns of this group,
    # rows 768:1152 = this group's w_proj rows padded to 1152 cols
    wq, wk, wv_ = (w_attn[:, k * D : (k + 1) * D] for k in range(3))
    wcomb_g = []
    for g in range(2):
        cols = slice(HDL * g, HDL * (g + 1))
        top = np.concatenate([wq[:, cols], wk[:, cols], wv_[:, cols]], axis=1)
        bot = np.zeros((HDL, WR), np.float32)
        bot[:, 0:D] = w_proj[HDL * g : HDL * (g + 1)]
        wcomb_g.append(np.concatenate([top, bot], axis=0).astype(bf16))

    in_maps = []
    for c in range(8):
        b, g, r = c // 2, c % 2, c // 2
        in_maps.append(
            {
                "xt": xtb[b][:, HL * g : HL * (g + 1)],
                "w": wcomb_g[g][288 * r : 288 * (r + 1)],
            }
        )
    return in_maps


def kernel(x, w_attn, w_proj, b_proj):
    nc = get_nc()
    in_maps = make_in_maps(x, w_attn, w_proj, b_proj)
    res = run_bass_kernel_spmd(nc, in_maps, core_ids=list(range(8)))
    out = np.empty((B, L, D), np.float32)
    for b in range(B):
        out[b, 0:HL] = np.asarray(res.results[2 * b]["out"])
        out[b, HL:L] = np.asarray(res.results[2 * b + 1]["out"])
    out += np.asarray(b_proj, dtype=np.float32)
    return out
